# revision 7
# baseline (speedup 1.0000x reference)
"""Trainium2 Bass kernel for nn_Phaseformer (32 conv branches + degenerate
single-token attention + unfold-mean pool), tensor-parallel over 8 NeuronCores.

Sharding: the 32 conv branches are packed into 16 balanced branch-pairs
(b, 31-b); each core owns 2 pairs (66 of the 528 concatenated T columns).
Every core runs the identical SPMD program; all per-branch heterogeneity
lives in host-prepared input data (weight slabs, im2col operands, masks).

The attention tail is linear in the per-core column slice, so each core
computes a [4,1] partial of the pooled output on-device; the host sums the
8 partials and broadcasts across the feature dim.  No device collective.

v3 vs v2 (trace-driven):
- bf16 GEMM operands (fp16 streamed at half PE rate: 282ns vs ~110ns / MM)
- 720KB weight DMAs (12 triggers instead of 22; ~650ns per trigger)
- conv bias folded into the PSUM accumulation via a K=2 matmul fed from a
  tiny early HWDGE constant (kills the zb pass; gelu reads PSUM directly)
- ACT tables preloaded via dummy activations (1.28us ACT_TABLE_LOAD was
  landing mid-chain)
- segmask pre-scaled by 1/(DIM*L) so the branch-sum matmul yields mu/E[y2]
  directly; stats chain cut to 5 ops + 2 fused STTs for u
"""

import os
import numpy as np
import ml_dtypes

import concourse.bass as bass
import concourse.tile as tile
import concourse.mybir as mybir
from concourse.alu_op_type import AluOpType
from concourse.bass_utils import run_bass_kernel_spmd

F32 = mybir.dt.float32
BF16 = mybir.dt.bfloat16
NPBF = ml_dtypes.bfloat16
AFT = mybir.ActivationFunctionType

N_CORES = 8
DUR = 32          # duration == number of branches
DIM = 256
T_TOTAL = DUR * (DUR + 1) // 2   # 528
K33 = 33          # taps per branch-pair (k_b + k_b' = 33)
CTRACT = K33 * DIM               # 8448 contraction length per pair GEMM
NCT = CTRACT // 128              # 66 contraction tiles
PAIRS_PER_CORE = 2
# pair-0 slab: [128, NCT*DIM] flat; chunk splits in tiles (256 cols each).
# Small first chunk -> earliest possible GEMM start.
P0_SPLIT = [4, 8, 12, 14, 14, 14]
# pair-1 slab is dout-split: all 66 tiles' cols 0:128 first, then cols
# 128:256, as 132 half-tiles of 128 cols; decreasing chunk sizes so the
# final transfer (and its completion latency) is small.
P1_SPLIT = [28, 28, 26, 22, 16, 12]
NCHUNK = 6
LN_EPS = 1e-5
N_W = 4           # pooled windows
POOL_STEP = 4 * DUR              # 128
S1_PAD = 640      # 528 padded to 5*128 for the tail matvec
NF = S1_PAD // 128               # 5

# c33h bf16 slab layout (per pair, stride C33P): lnw | lnb | wv
C33P = 2 * DIM + S1_PAD          # 1152
OFF_LNW, OFF_LNB, OFF_WV = 0, DIM, 2 * DIM

# c2e bf16 early slab [2, 578]: bias2 per pair (2x256) | segmT16 per pair
E_BIAS, E_SEGT = 0, 2 * DIM      # bias: P*256 within 0:512; segT: 512+P*33
CE = 2 * DIM + 2 * K33           # 578

# cf fp32 slab [128, CF]: segmask_scaled(2x2) | bv(5) | winm(4) |
# rows 0:2 -> segmT fp32 (33 cols x 2 pairs)
CF_SEG, CF_BV, CF_WINM, CF_SEGT = 0, 4, 9, 13
CF = 79

# c128h bf16 slab [128, 768]: wout_tr(640) | opb row (128 cols, row 0)
C128 = S1_PAD + 128

LAST_EXEC_TIME_NS = None
LAST_TRACE_DIR = None

_PROGRAM_CACHE = {}


# --------------------------------------------------------------------------
# axon NTFF profiling hook (used only when tracing is requested)
# --------------------------------------------------------------------------
def _install_ntff_hook():
    import sys, types, ctypes, contextlib
    if 'antenv.axon_hooks' in sys.modules:
        return
    try:
        mod = types.ModuleType('antenv.axon_hooks')
        _state = {}
        mod.set_axon_ntff_profile_hook = lambda h: _state.__setitem__('h', h)
        mod.get_axon_ntff_profile_hook = lambda: _state.get('h')
        sys.modules['antenv.axon_hooks'] = mod
        import antenv
        antenv.axon_hooks = mod

        so_path = '/opt/axon/libaxon_pjrt.so'
        lib = ctypes.CDLL(so_path)
        if not hasattr(lib, 'axon_start_nrt_profile'):
            return
        lib.axon_start_nrt_profile.argtypes = [ctypes.POINTER(ctypes.c_int64),
                                               ctypes.c_size_t]
        lib.axon_start_nrt_profile.restype = ctypes.c_int64
        lib.axon_stop_nrt_profile.argtypes = [ctypes.c_char_p]
        lib.axon_stop_nrt_profile.restype = ctypes.c_int64

        @contextlib.contextmanager
        def _hook(output_dir, device_ids):
            import jax
            jax.devices()
            if device_ids:
                ids = (ctypes.c_int64 * len(device_ids))(*device_ids)
                rc = lib.axon_start_nrt_profile(ids, len(device_ids))
            else:
                rc = lib.axon_start_nrt_profile(None, 0)
            if rc != 0:
                raise RuntimeError(f'axon_start_nrt_profile rc={rc}')
            try:
                yield
            finally:
                n = lib.axon_stop_nrt_profile(str(output_dir).encode())
                print(f'ntff profile: {n} file(s) -> {output_dir}')

        mod.set_axon_ntff_profile_hook(_hook)

        import concourse.bass_utils as bu
        bu.upload_artifacts = lambda tmpdir: f'file://{tmpdir}'
    except Exception as e:  # profiling is best-effort
        print(f'ntff hook install failed: {e}')


# --------------------------------------------------------------------------
# walrus here encodes at most ONE sem wait per instruction; split excess
# waits onto same-engine NoOps inserted just before the instruction.
# --------------------------------------------------------------------------
def _split_excess_waits(nc, max_waits=1):
    for fn in nc.m.functions:
        for bb in fn.blocks:
            new_list = []
            for ins in bb.instructions:
                si = ins.sync_info
                if si is not None and si.on_wait and len(si.on_wait) > max_waits:
                    waits = list(si.on_wait)
                    chunks = [waits[i:i + max_waits]
                              for i in range(0, len(waits), max_waits)]
                    for chunk in chunks[:-1]:
                        nop = mybir.InstNoOp(
                            name=nc.get_next_instruction_name(),
                            engine=ins.engine,
                            sync_info=mybir.SyncInfo(on_wait=list(chunk),
                                                     on_update=[]),
                        )
                        nc.register_instruction(nop)
                        new_list.append(nop)
                    si.on_wait = list(chunks[-1])
                new_list.append(ins)
            bb.instructions[:] = new_list


# --------------------------------------------------------------------------
# pairing / column-map helpers (shapes are structural constants)
# --------------------------------------------------------------------------
def _pair_info(p):
    """Pair p packs branches (b, b') = (p, 31-p): k=b+1 taps, L=32-b cols."""
    b, bp = p, 31 - p
    k, kp = b + 1, bp + 1        # k + kp = 33
    L, Lp = DUR - b, DUR - bp    # L + Lp = 33
    return b, bp, k, kp, L, Lp


def _branch_offset(b):
    # start of branch b inside the reference concat T axis
    return DUR * b - (b * (b - 1)) // 2


# --------------------------------------------------------------------------
# device program (built once, shared by all cores)
# --------------------------------------------------------------------------
def _build_program():
    nc = bass.Bass(trn_type="TRN2", target_bir_lowering=False,
                   num_devices=N_CORES)

    w0 = nc.declare_dram_parameter("w0", [128, NCT * DIM], BF16, isOutput=False)
    w1 = nc.declare_dram_parameter("w1", [128, 2 * NCT * 128], BF16,
                                   isOutput=False)
    xislab = nc.declare_dram_parameter("xislab", [PAIRS_PER_CORE, 128, NCT * K33],
                                       BF16, isOutput=False)
    c2e = nc.declare_dram_parameter("c2e", [2, CE], BF16, isOutput=False)
    c33h = nc.declare_dram_parameter("c33h", [K33, PAIRS_PER_CORE * C33P], BF16,
                                     isOutput=False)
    c128h = nc.declare_dram_parameter("c128h", [128, C128], BF16, isOutput=False)
    cf = nc.declare_dram_parameter("cf", [128, CF], F32, isOutput=False)
    out = nc.declare_dram_parameter("out", [N_W, 1], F32, isOutput=True)

    with tile.TileContext(nc) as tc:
        with tc.tile_pool(name="const", bufs=1) as const, \
             tc.tile_pool(name="zpool", bufs=1, space="PSUM") as zpool, \
             tc.tile_pool(name="spsum", bufs=1, space="PSUM") as spsum, \
             tc.tile_pool(name="qpsum", bufs=1, space="PSUM") as qpsum, \
             tc.tile_pool(name="fpsum", bufs=1, space="PSUM") as fpsum, \
             tc.tile_pool(name="work", bufs=2) as work:

            xi_sb = [const.tile([128, NCT * K33], BF16, name=f"xi{P}",
                                tag=f"xi{P}")
                     for P in range(PAIRS_PER_CORE)]
            c2e_sb = const.tile([2, CE], BF16, tag="c2e")
            c33_sb = const.tile([K33, PAIRS_PER_CORE * C33P], BF16, tag="c33")
            c128_sb = const.tile([128, C128], BF16, tag="c128")
            cf_sb = const.tile([128, CF], F32, tag="cf")
            c32_sb = const.tile([1, 1], BF16, tag="c32")
            magic_sb = const.tile([2, 1], F32, tag="magic")
            dumm_sb = const.tile([1, 4], F32, tag="dumm")
            u_sb = [const.tile([K33, 1], BF16, name=f"u{P}", tag=f"u{P}")
                    for P in range(PAIRS_PER_CORE)]

            # per-chunk weight tiles (chunk sizes vary; each gets its own
            # persistent buffer from the const pool)
            p0_off = np.cumsum([0] + P0_SPLIT).tolist()
            p1_off = np.cumsum([0] + P1_SPLIT).tolist()
            wt0 = [const.tile([128, P0_SPLIT[c] * DIM], BF16,
                              name=f"wt0_{c}", tag=f"w0_{c}")
                   for c in range(NCHUNK)]
            wt1 = [const.tile([128, P1_SPLIT[c] * 128], BF16,
                              name=f"wt1_{c}", tag=f"w1_{c}")
                   for c in range(NCHUNK)]

            # ---- DMA schedule -------------------------------------------
            # All triggers issue up front so both HWDGE rings stay busy
            # end-to-end; each trigger costs ~650ns (128 descriptors), so
            # the weight stream uses 12 x 720KB transfers.  Small constants
            # ride the otherwise-idle SWDGE (gpsimd) queue; the bias/segT
            # slab rides first on sync so the K=2 bias matmuls never stall
            # the accumulation start.
            def dma_c0(c, eng):
                eng.dma_start(wt0[c][:],
                              w0[:, p0_off[c] * DIM:p0_off[c + 1] * DIM])

            def dma_c1(c, eng):
                eng.dma_start(wt1[c][:],
                              w1[:, p1_off[c] * 128:p1_off[c + 1] * 128])

            # Ring assignment: BOTH rings carry pair-0's operands first
            # (its GEMM+stats must clear the PE/DVE streams well before
            # pair-1's half-B work arrives), pair-1 strictly after.  Bytes
            # per ring balanced to ~2.4MB (pair 0) + ~2.4MB (pair 1).
            XI_SPL = 16 * K33  # first 16 windows cover pair-0 chunks 0-1
            nc.sync.dma_start(xi_sb[0][:, 0:XI_SPL], xislab[0][:, 0:XI_SPL])
            dma_c0(0, nc.scalar)
            nc.sync.dma_start(c2e_sb[:], c2e[:])
            nc.scalar.dma_start(xi_sb[0][:, XI_SPL:], xislab[0][:, XI_SPL:])
            nc.gpsimd.dma_start(c33_sb[:], c33h[:])
            nc.gpsimd.dma_start(cf_sb[:], cf[:])
            nc.gpsimd.dma_start(c128_sb[:], c128h[:])

            nc.vector.memset(c32_sb[:], float(DIM) / N_CORES)
            # f32 whose bit pattern is 0x5f3759df (fast-inverse-sqrt magic)
            nc.vector.memset(magic_sb[:], 1.3211836172961055e+19)

            # preload the ACT table off the critical path (the table load
            # on first use of an activation function costs ~1.3us); issued
            # after two scalar-ring triggers so the weight stream starts
            # first, before the remaining triggers so ACT work never queues
            # behind ring-slot waits longer than necessary.
            nc.scalar.activation(dumm_sb[:, 0:1], c32_sb[:], AFT.Gelu)
            nc.scalar.activation(dumm_sb[:, 1:2], c32_sb[:], AFT.Square)

            dma_c0(1, nc.sync)
            dma_c0(3, nc.scalar)
            dma_c0(2, nc.sync)
            dma_c0(5, nc.scalar)
            dma_c0(4, nc.sync)
            nc.sync.dma_start(xi_sb[1][:], xislab[1])
            dma_c1(0, nc.scalar)
            dma_c1(1, nc.sync)
            dma_c1(2, nc.scalar)
            dma_c1(3, nc.sync)
            dma_c1(4, nc.scalar)
            dma_c1(5, nc.sync)

            # per-pair constant views inside c33h
            def c33(P, off, n):
                return c33_sb[:, P * C33P + off:P * C33P + off + n]

            vq = qpsum.tile([128, NF], F32, tag="vq")

            def bias_mm(zp_ap, P, d0, d1, stop):
                # conv bias accumulated last via a K=2 matmul segT16^T@bias2
                nc.tensor.matmul(
                    zp_ap,
                    lhsT=c2e_sb[0:2, E_SEGT + P * K33:E_SEGT + (P + 1) * K33],
                    rhs=c2e_sb[0:2,
                               E_BIAS + P * DIM + d0:E_BIAS + P * DIM + d1],
                    start=False, stop=stop)

            def elemwise(zp_ap, P, d0, d1, s0, s1, s2):
                # g = gelu(z) from PSUM with fused column sum; sum(g^2) via
                # ACT square; sum(g*lnw) via an STT with fused accumulate.
                n = d1 - d0
                g = work.tile([K33, n], BF16, tag=f"g{P}_{d0}")
                nc.scalar.activation(g[:], zp_ap, AFT.Gelu, accum_out=s0)
                sq = work.tile([K33, n], BF16, tag=f"sq{P}_{d0}")
                nc.scalar.activation(sq[:], g[:], AFT.Square, accum_out=s1)
                glnw = work.tile([K33, n], BF16, tag=f"gl{P}_{d0}")
                nc.vector.scalar_tensor_tensor(
                    out=glnw[:], in0=g[:], scalar=1.0,
                    in1=c33(P, OFF_LNW + d0, n),
                    op0=AluOpType.mult, op1=AluOpType.mult, accum_out=s2)

            def stats_chain(P, stk01, stk2, stk34):
                # branch stats: bst = scaled_segmask^T @ [stk0 stk1]
                #   -> row s: [mu_s, E[y^2]_s]  (mask pre-scaled by 1/(DIM*L))
                # rstd = 1/sqrt(var) via the fast-inverse-sqrt bit trick + one
                # Newton step, entirely on DVE: no Sqrt on ACT (its table
                # cannot coexist with Gelu's; each switch is a 1.3us reload),
                # and eps=1e-5 is dropped (var >= 1e-2 here; 5e-6 relative).
                st = work.tile([2, 8], F32, tag="st")
                stu = st[:].bitcast(mybir.dt.uint32)
                bst = spsum.tile([2, 2], F32, tag="bst")
                nc.tensor.matmul(
                    bst[:],
                    lhsT=cf_sb[0:K33, CF_SEG + 2 * P:CF_SEG + 2 * P + 2],
                    rhs=stk01, start=True, stop=True)
                nc.scalar.activation(st[:, 0:1], bst[:, 0:1], AFT.Square)
                nc.vector.scalar_tensor_tensor(
                    out=st[:, 1:2], in0=st[:, 0:1], scalar=-1.0,
                    in1=bst[:, 1:2], op0=AluOpType.mult, op1=AluOpType.add)
                nc.vector.tensor_scalar(
                    stu[:, 2:3], stu[:, 1:2], 1, None,
                    AluOpType.logical_shift_right)               # i >> 1
                nc.vector.tensor_tensor(
                    stu[:, 3:4], magic_sb[:].bitcast(mybir.dt.uint32),
                    stu[:, 2:3], AluOpType.subtract)             # y0 bits
                nc.vector.scalar_tensor_tensor(
                    out=st[:, 4:5], in0=st[:, 3:4], scalar=st[:, 3:4],
                    in1=st[:, 1:2], op0=AluOpType.mult,
                    op1=AluOpType.mult)                          # v*y0^2
                nc.vector.tensor_scalar(st[:, 5:6], st[:, 4:5], -0.5, 1.5,
                                        AluOpType.mult, AluOpType.add)
                mr = work.tile([2, 2], F32, tag="mr")
                nc.vector.tensor_tensor(mr[:, 0:1], st[:, 3:4], st[:, 5:6],
                                        AluOpType.mult)          # rstd
                nc.vector.tensor_tensor(mr[:, 1:2], mr[:, 0:1], bst[:, 0:1],
                                        AluOpType.mult)          # rstd*mu

                # broadcast branch scalars to the 33 columns
                bc = spsum.tile([K33, 2], F32, tag="bc")
                nc.tensor.matmul(
                    bc[:],
                    lhsT=cf_sb[0:2, CF_SEGT + K33 * P:CF_SEGT + K33 * (P + 1)],
                    rhs=mr[:], start=True, stop=True)

                # u = rstd*cs_glnw - (rstd*mu)*cs_lnw + cs_lnb, as two fused
                # STTs with the bc columns as per-partition scalars
                t1 = work.tile([K33, 1], F32, tag="t1")
                nc.vector.scalar_tensor_tensor(
                    out=t1[:], in0=stk34[:, 0:1], scalar=bc[:, 1:2],
                    in1=stk34[:, 1:2], op0=AluOpType.mult,
                    op1=AluOpType.subtract)
                nc.vector.scalar_tensor_tensor(
                    out=u_sb[P][:], in0=stk2, scalar=bc[:, 0:1],
                    in1=t1[:], op0=AluOpType.mult, op1=AluOpType.subtract)

                # partial of q: vq[:, f] += wv_f^T @ u  (pair 0's matvec
                # runs during pair 1's GEMM stream)
                for f in range(NF):
                    nc.tensor.matmul(
                        vq[:, f:f + 1],
                        lhsT=c33(P, OFF_WV + f * 128, 128),
                        rhs=u_sb[P][:],
                        start=(P == 0 and f == 0),
                        stop=(P == PAIRS_PER_CORE - 1 and f == NF - 1))

            def lnw_sums(P, stk34):
                # sum(lnw), sum(lnb): constants only; issued early so they
                # run during the GEMM stream
                nc.vector.tensor_reduce(stk34[:, 0:1], c33(P, OFF_LNW, DIM),
                                        mybir.AxisListType.X, AluOpType.add)
                nc.vector.tensor_reduce(stk34[:, 1:2], c33(P, OFF_LNB, DIM),
                                        mybir.AxisListType.X, AluOpType.add)

            # ---- pair 0: plain [33,256] GEMM ----------------------------
            zp0 = zpool.tile([K33, DIM], F32, tag="z0")
            for c in range(NCHUNK):
                for jj in range(P0_SPLIT[c]):
                    j = p0_off[c] + jj
                    nc.tensor.matmul(
                        zp0[:],
                        lhsT=xi_sb[0][:, j * K33:(j + 1) * K33],
                        rhs=wt0[c][:, jj * DIM:(jj + 1) * DIM],
                        start=(j == 0), stop=False)
            bias_mm(zp0[:], 0, 0, DIM, stop=True)

            stk0 = work.tile([K33, 4], F32, tag="stk0")
            stk34_0 = work.tile([K33, 2], F32, tag="stk34_0")
            lnw_sums(0, stk34_0)
            elemwise(zp0[:], 0, 0, DIM,
                     stk0[:, 0:1], stk0[:, 1:2], stk0[:, 2:3])
            stats_chain(0, stk0[:, 0:2], stk0[:, 2:3], stk34_0)

            # ---- pair 1: dout-split GEMM (cols 0:128, then 128:256) -----
            # half A's stats run while half B's weights stream; only half
            # B's elementwise work remains after the last transfer.
            zpA = zpool.tile([K33, 128], F32, tag="zA")
            zpB = zpool.tile([K33, 128], F32, tag="zB")
            stkA = work.tile([K33, 4], F32, tag="stkA")
            stkB = work.tile([K33, 4], F32, tag="stkB")
            stkS = work.tile([K33, 4], F32, tag="stkS")
            stk34_1 = work.tile([K33, 2], F32, tag="stk34_1")
            lnw_sums(1, stk34_1)
            for c in range(NCHUNK):
                for ss in range(P1_SPLIT[c]):
                    ht = p1_off[c] + ss
                    h, j = (0, ht) if ht < NCT else (1, ht - NCT)
                    zx = zpA if h == 0 else zpB
                    nc.tensor.matmul(
                        zx[:],
                        lhsT=xi_sb[1][:, j * K33:(j + 1) * K33],
                        rhs=wt1[c][:, ss * 128:(ss + 1) * 128],
                        start=(j == 0), stop=False)
                    if ht == NCT - 1:
                        bias_mm(zpA[:], 1, 0, 128, stop=True)
                        elemwise(zpA[:], 1, 0, 128, stkA[:, 0:1],
                                 stkA[:, 1:2], stkA[:, 2:3])
            bias_mm(zpB[:], 1, 128, DIM, stop=True)
            elemwise(zpB[:], 1, 128, DIM,
                     stkB[:, 0:1], stkB[:, 1:2], stkB[:, 2:3])
            nc.vector.tensor_tensor(stkS[:, 0:2], stkA[:, 0:2], stkB[:, 0:2],
                                    AluOpType.add)
            nc.vector.tensor_tensor(stkS[:, 2:3], stkA[:, 2:3], stkB[:, 2:3],
                                    AluOpType.add)
            stats_chain(1, stkS[:, 0:2], stkS[:, 2:3], stk34_1)

            # ---- attention tail (all partial w.r.t. this core) ----------
            s1 = work.tile([128, NF], BF16, tag="s1")
            nc.vector.scalar_tensor_tensor(
                out=s1[:], in0=cf_sb[:, CF_BV:CF_BV + NF],
                scalar=float(DIM) / N_CORES,
                in1=vq[:], op0=AluOpType.mult, op1=AluOpType.add)

            # v = Wout_sel @ q + (DIM/N_CORES) * opb_sel, accumulated in PSUM
            vps = fpsum.tile([128, 1], F32, tag="vps")
            for f in range(NF):
                nc.tensor.matmul(vps[:],
                                 lhsT=c128_sb[:, f * 128:(f + 1) * 128],
                                 rhs=s1[:, f:f + 1],
                                 start=(f == 0), stop=False)
            nc.tensor.matmul(vps[:], lhsT=c128_sb[0:1, S1_PAD:S1_PAD + 128],
                             rhs=c32_sb[:], start=False, stop=True)

            s2 = work.tile([128, 1], F32, tag="s2")
            nc.vector.tensor_copy(s2[:], vps[:])

            # window-mean pooling of the 128 selected rows
            ops = fpsum.tile([N_W, 1], F32, tag="ops")
            nc.tensor.matmul(ops[:], lhsT=cf_sb[:, CF_WINM:CF_WINM + N_W],
                             rhs=s2[:], start=True, stop=True)
            p4 = work.tile([N_W, 1], F32, tag="p4")
            nc.vector.tensor_copy(p4[:], ops[:])
            nc.sync.dma_start(out[:], p4[:])

    _split_excess_waits(nc)
    return nc


# --------------------------------------------------------------------------
# host-side sharding (indexing / gather / transpose / zero-fill / cast only)
# --------------------------------------------------------------------------
def _host_prepare(inputs):
    x = np.ascontiguousarray(inputs["x"], dtype=np.float32)
    conv_w = np.asarray(inputs["conv_w"], dtype=np.float32)
    conv_b = np.asarray(inputs["conv_b"], dtype=np.float32)
    ln_w = np.asarray(inputs["ln_w"], dtype=np.float32)
    ln_b = np.asarray(inputs["ln_b"], dtype=np.float32)
    in_proj_w = np.asarray(inputs["in_proj_w"], dtype=np.float32)
    in_proj_b = np.asarray(inputs["in_proj_b"], dtype=np.float32)
    out_proj_w = np.asarray(inputs["out_proj_w"], dtype=np.float32)
    out_proj_b = np.asarray(inputs["out_proj_b"], dtype=np.float32)

    xt = np.ascontiguousarray(x[0].T)            # (DIM, DUR)
    Wv = in_proj_w[2 * T_TOTAL:]                 # (T, T) value slice
    bv = in_proj_b[2 * T_TOTAL:]                 # (T,)

    # shared (core-independent) tensors -----------------------------------
    row_sel = np.asarray([POOL_STEP * w + j
                          for w in range(N_W) for j in range(DUR)])

    cf = np.zeros((128, CF), np.float32)
    bv_flat = np.zeros(S1_PAD, np.float32)
    bv_flat[:T_TOTAL] = bv
    cf[:, CF_BV:CF_BV + NF] = bv_flat.reshape(NF, 128).T
    for o in range(128):
        cf[o, CF_WINM + o // DUR] = 1.0 / DUR

    m = np.zeros((S1_PAD, 128), np.float32)
    m[:T_TOTAL, :] = out_proj_w[row_sel].T       # [s1_idx, sel_row]
    wout_tr = (m.reshape(NF, 128, 128).transpose(1, 0, 2)
                .reshape(128, S1_PAD))
    c128h = np.zeros((128, C128), NPBF)
    c128h[:, :S1_PAD] = wout_tr.astype(NPBF)
    c128h[0, S1_PAD:S1_PAD + 128] = out_proj_b[row_sel].astype(NPBF)

    in_maps = []
    for core in range(N_CORES):
        wslab = np.empty((PAIRS_PER_CORE, K33, DIM, DIM), np.float32)
        xisl = np.zeros((PAIRS_PER_CORE, K33, DIM, K33), np.float32)
        c33 = np.zeros((K33, PAIRS_PER_CORE * C33P), np.float32)
        c2e = np.zeros((2, CE), np.float32)
        cfc = cf.copy()
        tmap = np.empty(PAIRS_PER_CORE * K33, np.int64)

        for Pl in range(PAIRS_PER_CORE):
            p = PAIRS_PER_CORE * core + Pl
            b, bp, k, kp, L, Lp = _pair_info(p)
            o = Pl * C33P

            # weight slab: taps [0,k) from branch b, taps [k,33) from b'
            wslab[Pl, :k] = conv_w[b, :, :, :k].transpose(2, 1, 0)
            wslab[Pl, k:] = conv_w[bp, :, :, :kp].transpose(2, 1, 0)

            # im2col: cols [0,L) use branch-b taps, cols [L,33) branch-b'
            for t in range(k):
                xisl[Pl, t, :, 0:L] = xt[:, t:t + L]
            for tl in range(kp):
                xisl[Pl, k + tl, :, L:K33] = xt[:, tl:tl + Lp]

            c33[0:L, o + OFF_LNW:o + OFF_LNW + DIM] = ln_w[b, :, :L].T
            c33[L:K33, o + OFF_LNW:o + OFF_LNW + DIM] = ln_w[bp, :, :Lp].T
            c33[0:L, o + OFF_LNB:o + OFF_LNB + DIM] = ln_b[b, :, :L].T
            c33[L:K33, o + OFF_LNB:o + OFF_LNB + DIM] = ln_b[bp, :, :Lp].T

            # early slab: conv bias rows + branch-segment indicator
            c2e[0, E_BIAS + Pl * DIM:E_BIAS + (Pl + 1) * DIM] = conv_b[b]
            c2e[1, E_BIAS + Pl * DIM:E_BIAS + (Pl + 1) * DIM] = conv_b[bp]
            c2e[0, E_SEGT + Pl * K33:E_SEGT + Pl * K33 + L] = 1.0
            c2e[1, E_SEGT + Pl * K33 + L:E_SEGT + (Pl + 1) * K33] = 1.0

            cfc[0:L, CF_SEG + 2 * Pl] = 1.0 / (DIM * L)
            cfc[L:K33, CF_SEG + 2 * Pl + 1] = 1.0 / (DIM * Lp)
            cfc[0, CF_SEGT + K33 * Pl:CF_SEGT + K33 * Pl + L] = 1.0
            cfc[1, CF_SEGT + K33 * Pl + L:CF_SEGT + K33 * (Pl + 1)] = 1.0

            tmap[Pl * K33:Pl * K33 + L] = _branch_offset(b) + np.arange(L)
            tmap[Pl * K33 + L:(Pl + 1) * K33] = _branch_offset(bp) + np.arange(Lp)

        for Pl in range(PAIRS_PER_CORE):
            o = Pl * C33P
            c33[:, o + OFF_WV:o + OFF_WV + T_TOTAL] = \
                Wv[:, tmap[Pl * K33:(Pl + 1) * K33]].T

        # pair-0 slab: [128, tile-major 256-col blocks]
        wtiles = wslab.reshape(PAIRS_PER_CORE, NCT, 128, DIM)
        w0 = wtiles[0].transpose(1, 0, 2).reshape(128, NCT * DIM)
        # pair-1 slab: dout-split into 132 half-tiles of 128 cols, all
        # cols 0:128 first, then cols 128:256
        w1 = (wtiles[1].reshape(NCT, 128, 2, 128)
              .transpose(2, 0, 1, 3)        # [half, tile, part, 128]
              .transpose(2, 0, 1, 3)        # [part, half, tile, 128]
              .reshape(128, 2 * NCT * 128))

        in_maps.append({
            "w0": np.ascontiguousarray(w0.astype(NPBF)),
            "w1": np.ascontiguousarray(w1.astype(NPBF)),
            "xislab": np.ascontiguousarray(
                xisl.reshape(PAIRS_PER_CORE, CTRACT, K33)
                    .reshape(PAIRS_PER_CORE, NCT, 128, K33)
                    .transpose(0, 2, 1, 3)
                    .reshape(PAIRS_PER_CORE, 128, NCT * K33)
                    .astype(NPBF)),
            "c2e": np.ascontiguousarray(c2e.astype(NPBF)),
            "c33h": np.ascontiguousarray(c33.astype(NPBF)),
            "c128h": c128h,
            "cf": np.ascontiguousarray(cfc),
        })
    return in_maps


def kernel(**inputs):
    global LAST_EXEC_TIME_NS, LAST_TRACE_DIR
    trace = bool(int(os.environ.get("KERNEL_TRACE", "0")))
    if trace:
        _install_ntff_hook()

    if "nc" not in _PROGRAM_CACHE:
        _PROGRAM_CACHE["nc"] = _build_program()
    nc = _PROGRAM_CACHE["nc"]

    in_maps = _host_prepare(inputs)

    kwargs = {}
    if trace:
        import tempfile
        LAST_TRACE_DIR = tempfile.mkdtemp(prefix="phaseformer_trace_")
        kwargs = dict(trace=True, tmpdir=LAST_TRACE_DIR)
    res = run_bass_kernel_spmd(nc, in_maps, list(range(N_CORES)), **kwargs)
    LAST_EXEC_TIME_NS = res.exec_time_ns

    acc = np.zeros((N_W, 1), np.float64)
    for i in range(N_CORES):
        acc += res.results[i]["out"].astype(np.float64)
    full = np.broadcast_to(acc.astype(np.float32).reshape(1, N_W, 1),
                           (1, N_W, DIM))
    return np.ascontiguousarray(full)
